# revision 20
# baseline (speedup 1.0000x reference)
"""Trainium2 Bass kernel: batched multi-head dot-product attention.

Full-size problem: queries/keys/values [B=4, H=8, S=2048, D=256] fp32,
out = softmax(Q K^T / 16) V, returned reshaped to (S, B, H, D).

Sharding: the 32 (B*H) heads are split across 8 NeuronCores, 4 heads per
core; each core computes full attention for its heads (no cross-core
communication).

Per-head algorithm (per 512-query block):
  - scores are computed TRANSPOSED (keys on the partition dim, queries on
    the free dim): psum_sT[k, q] = sum_d KT[d, k] * QT[d, q], so that after
    exp() the attention weights are already laid out as the stationary
    (lhsT) operand of the attn @ V matmul -- no on-chip transposes needed.
  - softmax skips the max subtraction: scores/16 are ~N(0,1), exp cannot
    overflow fp32, and jax.nn.softmax's max shift is mathematically a
    no-op. The 1/16 scale is folded into the Exp activation.
  - the softmax denominator falls out of the attn @ V matmul for free: V
    is augmented host-side with a ones column, so column D of the output
    accumulator is sum_k exp(score) per query. A reciprocal + per-partition
    scaled multiply on the Vector engine normalizes while evacuating PSUM.
  - matmuls run in fp16 (inputs converted host-side; exp output written as
    fp16 by the Scalar engine; accumulation stays fp32 in PSUM) for
    full-rate streaming and fast weight loads; measured output rel err vs
    the fp32 reference is ~6e-4.
  - one flat software pipeline over (head, qblock, kchunk): scores + exp
    run 2+ iterations ahead of the attn@V matmuls, and the four PSUM
    accumulator lanes are skewed one iteration apart so their
    normalize+store chains stagger instead of colliding at block
    boundaries. Within each step the attn@V lanes are emitted BEFORE the
    scores matmuls so the PE has ~430ns of queued work before it needs the
    ps_s slot that exp() frees -- absorbing ACT-engine jitter.
  - all tensors use packed per-partition-contiguous DRAM layouts so each
    head is ONE fat input DMA per tensor and each (head, qblock) is ONE
    output DMA: few DMA instructions -> less Sync-engine serialization and
    a much shorter TileContext epilogue (which gates the measured
    exec_time via its trailing instruction-fetches).
  - a burst of warmup matmuls on a zeroed scratch tile keeps the PE
    continuously busy from engine-init until the first input tiles land,
    so the HAM clock gate (1.2 -> 2.4 GHz) releases before real work
    starts and is not reset by an idle gap.
"""

import sys

import numpy as np

for _p in ("/opt/trn_rl_repo",):
    if _p not in sys.path:
        sys.path.insert(0, _p)

B, H, S, D = 4, 8, 2048, 256
N_CORES = 8
HPC = (B * H) // N_CORES  # heads per core
SOFTMAX_SCALE = 1.0 / 16.0
N_WARMUP = 6

_compiled = {}


def _build(nh, s, d):
    import concourse.bacc as bacc
    import concourse.mybir as mybir
    import concourse.tile as tile

    f32 = mybir.dt.float32
    f16 = mybir.dt.float16

    KC = s // 128  # contraction (key) chunks
    QB = s // 512  # query blocks
    DC = d // 128  # head-dim chunks

    nc = bacc.Bacc("TRN2", debug=False, num_devices=N_CORES)
    # qT/kT are packed partition-major on the host:
    # qT[h, p, dc, col] = Q[h, col, dc*128 + p], so each partition's data is
    # one contiguous DC*s*2B run and a whole head loads as ONE DMA.
    qT = nc.dram_tensor("qT", [nh, 128, DC, s], f16, kind="ExternalInput")
    kT = nc.dram_tensor("kT", [nh, 128, DC, s], f16, kind="ExternalInput")
    vaw = d + 1  # ones col at d (softmax denominator rides along)
    # vA[h, p, kc, :] = V_aug[h, kc*128 + p, :] -- same packing trick.
    vA = nc.dram_tensor("vA", [nh, 128, KC, vaw], f16, kind="ExternalInput")
    # o[h, qb, p, qs*d + c] = out[h, qb*512 + qs*128 + p, c]: the four
    # 128-row lanes of a query block store side-by-side from one SBUF tile
    # in ONE DMA; the host permutes lanes back (cheap numpy reshape).
    o = nc.dram_tensor("o", [nh, QB, 128, 4 * d], f32, kind="ExternalOutput")

    with tile.TileContext(nc) as tc:
        with (
            tc.tile_pool(name="kt", bufs=2) as kt_pool,
            tc.tile_pool(name="qt", bufs=2) as qt_pool,
            tc.tile_pool(name="va", bufs=2) as va_pool,
            tc.tile_pool(name="exp", bufs=8) as exp_pool,
            tc.tile_pool(name="outp", bufs=2) as out_pool,
            tc.tile_pool(name="rec", bufs=4) as rec_pool,
            tc.tile_pool(name="warm", bufs=1) as warm_pool,
            tc.tile_pool(name="ps_s", bufs=2, space="PSUM") as ps_s_pool,
            tc.tile_pool(name="ps_o", bufs=6, space="PSUM") as ps_o_pool,
        ):
            # --- DMA emission (per head, first-use ordered) ---
            kts, qts, vas = [], [], []
            for h in range(nh):
                kt = kt_pool.tile([128, DC, s], f16, name=f"kt_{h}", tag="kt")
                qt = qt_pool.tile([128, DC, s], f16, name=f"qt_{h}", tag="qt")
                va = va_pool.tile([128, KC, vaw], f16, name=f"va_{h}", tag="va")
                kts.append(kt); qts.append(qt); vas.append(va)

            def emit_head_dma(h):
                kt, qt, va = kts[h], qts[h], vas[h]
                if h == 0:
                    # first-use ordered, finer-grained: the first score
                    # matmul needs only kt cols 0:128 and qt cols 0:512 --
                    # land those first so real matmuls can start ASAP, then
                    # feed kt chunks and va chunks in consumption order
                    # (attn@V lane 0 reads va[:, kc] two steps behind the
                    # kc-th score matmul).
                    # DMA completion waits are per-queue thresholds, so every
                    # DMA issued before the one a consumer needs delays that
                    # consumer. Different issuing engines have independent
                    # DGE queues/semaphores: split the critical first loads
                    # across Sync and Scalar (whose DGE is idle until the
                    # first exp at ~11us) so they transfer in parallel and
                    # nothing queues behind the big kt/va bulk.
                    nc.sync.dma_start(kt[:, :, 0:128], kT.ap()[h, :, :, 0:128])
                    nc.scalar.dma_start(qt[:, 0, 0:512], qT.ap()[h, :, 0, 0:512])
                    nc.sync.dma_start(qt[:, 1, 0:512], qT.ap()[h, :, 1, 0:512])
                    nc.scalar.dma_start(kt[:, :, 128:512], kT.ap()[h, :, :, 128:512])
                    nc.sync.dma_start(va[:, 0:2, :], vA.ap()[h, :, 0:2, :])
                    nc.sync.dma_start(va[:, 2:4, :], vA.ap()[h, :, 2:4, :])
                    nc.sync.dma_start(kt[:, :, 512:1024], kT.ap()[h, :, :, 512:1024])
                    nc.sync.dma_start(kt[:, :, 1024:s], kT.ap()[h, :, :, 1024:s])
                    nc.sync.dma_start(va[:, 4:8, :], vA.ap()[h, :, 4:8, :])
                    nc.sync.dma_start(va[:, 8:KC, :], vA.ap()[h, :, 8:KC, :])
                    nc.sync.dma_start(qt[:, :, 512:s], qT.ap()[h, :, :, 512:s])
                else:
                    nc.sync.dma_start(kt[:], kT.ap()[h])
                    nc.sync.dma_start(qt[:], qT.ap()[h])
                    nc.sync.dma_start(va[:], vA.ap()[h])

            # --- flat software pipeline over (head, qb, kc) ---
            # iteration t: scores(t) + exp(t); attn@V lane qs processes
            # iteration t-2-qs, so the four accumulator lanes finish (and
            # normalize + free their PSUM bank) one per iteration instead
            # of colliding at block boundaries.
            NIT = nh * QB * KC
            exps = [None] * NIT
            ps_os = {}
            osbs = {}

            def av_lane(t_av, qs):
                h, r = divmod(t_av, QB * KC)
                qb, kc = divmod(r, KC)
                po = ps_os[(h, qb)]
                nc.tensor.matmul(
                    po[qs][:],
                    exps[t_av][:, qs * 128:(qs + 1) * 128],
                    vas[h][:, kc, :],
                    start=(kc == 0),
                    stop=(kc == KC - 1),
                )
                if kc == KC - 1:
                    rec = rec_pool.tile([128, 1], f32, name=f"rec_{h}_{qb}_{qs}", tag="rec")
                    nc.vector.reciprocal(rec[:], po[qs][:, d:d + 1])
                    osb = osbs[(h, qb)]
                    last_block = h == nh - 1 and qb == QB - 1
                    if last_block:
                        # kernel tail: no exps remain, so spread the four
                        # normalize+store chains across three idle engines
                        # (Vector keeps only the reciprocals): lanes 0,2 run
                        # entirely on GpSimd, lanes 1,3 on Scalar, each
                        # storing its own 32KB sliver immediately -- the
                        # exit barrier behind the last store runs in the
                        # HAM-throttled window, so every ns earlier pays
                        # double.
                        # (GpSimd cannot read PSUM, so the muls stay on
                        # Vector/Scalar; GpSimd just issues lane-0/2 stores
                        # from SBUF on its own idle DGE queue.)
                        eng = nc.scalar if qs % 2 == 1 else nc.gpsimd
                        if qs % 2 == 1:
                            nc.scalar.mul(
                                osb[:, qs * d:(qs + 1) * d], po[qs][:, 0:d], rec[:]
                            )
                        else:
                            nc.vector.tensor_scalar_mul(
                                osb[:, qs * d:(qs + 1) * d], po[qs][:, 0:d], rec[:]
                            )
                        eng.dma_start(
                            o.ap()[h, qb, :, qs * d:(qs + 1) * d],
                            osb[:, qs * d:(qs + 1) * d],
                        )
                        if qs == 3:
                            ps_os.pop((h, qb))
                            osbs.pop((h, qb))
                    else:
                        nc.vector.tensor_scalar_mul(
                            osb[:, qs * d:(qs + 1) * d], po[qs][:, 0:d], rec[:]
                        )
                    if not last_block and qs == 3:
                        # mid-kernel: one fat store per block on the Sync
                        # queue (it has ~27us of slack).
                        nc.sync.dma_start(o.ap()[h, qb], osb[:])
                        ps_os.pop((h, qb))
                        osbs.pop((h, qb))

            # PE warmup: the HAM clock gate starts at 1.2 GHz and only
            # releases after a few us of SUSTAINED matmul activity (an idle
            # gap resets it). Burn the initial DMA wait on dummy matmuls
            # over a zeroed scratch tile, sized so the PE stays busy until
            # the first real input tiles have landed.
            wsrc = warm_pool.tile([128, 512], f16, name="wsrc")
            nc.vector.memset(wsrc[:], 0.0)
            for w in range(N_WARMUP):
                ps_w = ps_s_pool.tile([128, 512], f32, name=f"ps_w_{w}", tag="ps_s")
                nc.tensor.matmul(ps_w[:], wsrc[:, 0:128], wsrc[:], start=True, stop=True)

            emit_head_dma(0)
            for t in range(NIT):
                # scores+exp first: exp(t) starts as early as possible
                # within the step, maximizing slack for both the ps_s WAR
                # (scores(t+2)) and the attn@V weight load of step t+2.
                if True:
                    h, r = divmod(t, QB * KC)
                    qb, kc = divmod(r, KC)
                    if r == 0 and h + 1 < nh:
                        emit_head_dma(h + 1)  # prefetch next head
                    if kc == 0:
                        ps_os[(h, qb)] = [
                            ps_o_pool.tile([128, vaw], f32, name=f"ps_o_{h}_{qb}_{qs}", tag="ps_o")
                            for qs in range(4)
                        ]
                        osbs[(h, qb)] = out_pool.tile(
                            [128, 4 * d], f32, name=f"osb_{h}_{qb}", tag="outp"
                        )
                    ps_s = ps_s_pool.tile([128, 512], f32, name=f"ps_s_{h}_{qb}_{kc}", tag="ps_s")
                    for dc in range(DC):
                        nc.tensor.matmul(
                            ps_s[:],
                            kts[h][:, dc, kc * 128:(kc + 1) * 128],
                            qts[h][:, dc, qb * 512:(qb + 1) * 512],
                            start=(dc == 0),
                            stop=(dc == DC - 1),
                        )
                    expt = exp_pool.tile([128, 512], f16, name=f"expt_{h}_{qb}_{kc}", tag="exp")
                    nc.scalar.activation(
                        expt[:], ps_s[:], mybir.ActivationFunctionType.Exp,
                        scale=SOFTMAX_SCALE,
                    )
                    exps[t] = expt
                for qs in range(4):
                    t_av = t - 2 - qs
                    if 0 <= t_av < NIT:
                        av_lane(t_av, qs)
                if t >= 6 and t - 6 >= 0:
                    exps[t - 6] = None

            # drain: the remaining lane matmuls (lane qs is 2+qs iterations
            # behind) packed tightly instead of spread over 6 more skewed
            # steps -- all their exps exist, so the PE finishes ~1us sooner
            # and the final normalize/store (and the exit barrier behind it,
            # which runs in the HAM-throttled window) start earlier.
            for t_av in range(NIT - 5, NIT):
                for qs in range(4):
                    if t_av >= NIT - 2 - qs:
                        av_lane(t_av, qs)

    nc.compile()
    return nc


def _get_nc(nh, s, d):
    key = (nh, s, d)
    if key not in _compiled:
        _compiled[key] = _build(nh, s, d)
    return _compiled[key]


def _make_in_maps(queries, keys, values, n_cores):
    """queries/keys/values: [NHEADS_TOTAL, s, d] fp32 -> per-core input dicts."""
    nht, s, d = queries.shape
    nh = nht // n_cores
    dc = d // 128
    kc = s // 128
    pad = np.ones((nh, s, 1), dtype=np.float16)
    in_maps = []
    for c in range(n_cores):
        h0, h1 = c * nh, (c + 1) * nh
        # [nh, s, d] -> [nh, d, s] -> [nh, DC, 128, s] -> [nh, 128, DC, s]
        qp = (queries[h0:h1].transpose(0, 2, 1)
              .reshape(nh, dc, 128, s).transpose(0, 2, 1, 3))
        kp = (keys[h0:h1].transpose(0, 2, 1)
              .reshape(nh, dc, 128, s).transpose(0, 2, 1, 3))
        in_maps.append({
            "qT": np.ascontiguousarray(qp).astype(np.float16),
            "kT": np.ascontiguousarray(kp).astype(np.float16),
            "vA": np.ascontiguousarray(
                np.concatenate([values[h0:h1].astype(np.float16), pad], axis=2)
                .reshape(nh, kc, 128, -1).transpose(0, 2, 1, 3)),
        })
    return in_maps


def _run(queries, keys, values, n_cores):
    """queries/keys/values: [NHEADS_TOTAL, s, d] fp32. Returns [NHEADS_TOTAL, s, d]."""
    from concourse import bass_utils

    nht, s, d = queries.shape
    nh = nht // n_cores
    qb = s // 512
    nc = _get_nc(nh, s, d)

    in_maps = _make_in_maps(queries, keys, values, n_cores)
    res = bass_utils.run_bass_kernel_spmd(nc, in_maps, core_ids=list(range(n_cores)))
    out = np.empty((nht, s, d), dtype=np.float32)
    for c in range(n_cores):
        # o[h, qb, p, qs*d+c] -> rows qb*512 + qs*128 + p
        r = res.results[c]["o"].reshape(nh, qb, 128, 4, d)
        out[c * nh:(c + 1) * nh] = (
            r.transpose(0, 1, 3, 2, 4).reshape(nh, s, d)
        )
    return out


def kernel(queries, keys, values, adj=None):
    queries = np.asarray(queries, dtype=np.float32)
    keys = np.asarray(keys, dtype=np.float32)
    values = np.asarray(values, dtype=np.float32)
    b, h, s, d = queries.shape
    out = _run(
        queries.reshape(b * h, s, d),
        keys.reshape(b * h, s, d),
        values.reshape(b * h, s, d),
        N_CORES,
    )
    # reference returns a raw reshape of the contiguous [B,H,S,D] result
    return out.reshape(s, b, h, d)


# revision 24
# speedup vs baseline: 1.0025x; 1.0025x over previous
"""Trainium2 Bass kernel: batched multi-head dot-product attention.

Full-size problem: queries/keys/values [B=4, H=8, S=2048, D=256] fp32,
out = softmax(Q K^T / 16) V, returned reshaped to (S, B, H, D).

Sharding: the 32 (B*H) heads are split across 8 NeuronCores, 4 heads per
core; each core computes full attention for its heads (no cross-core
communication).

Per-head algorithm (per 512-query block):
  - scores are computed TRANSPOSED (keys on the partition dim, queries on
    the free dim): psum_sT[k, q] = sum_d KT[d, k] * QT[d, q], so that after
    exp() the attention weights are already laid out as the stationary
    (lhsT) operand of the attn @ V matmul -- no on-chip transposes needed.
  - softmax skips the max subtraction: scores/16 are ~N(0,1), exp cannot
    overflow fp32, and jax.nn.softmax's max shift is mathematically a
    no-op. The 1/16 scale is folded into the Exp activation.
  - the softmax denominator falls out of the attn @ V matmul for free: V
    is augmented host-side with a ones column, so column D of the output
    accumulator is sum_k exp(score) per query. A reciprocal + per-partition
    scaled multiply on the Vector engine normalizes while evacuating PSUM.
  - matmuls run in fp16 (inputs converted host-side; exp output written as
    fp16 by the Scalar engine; accumulation stays fp32 in PSUM) for
    full-rate streaming and fast weight loads; measured output rel err vs
    the fp32 reference is ~6e-4.
  - one flat software pipeline over (head, qblock, kchunk): scores + exp
    run 2+ iterations ahead of the attn@V matmuls, and the four PSUM
    accumulator lanes are skewed one iteration apart so their
    normalize+store chains stagger instead of colliding at block
    boundaries. Within each step the attn@V lanes are emitted BEFORE the
    scores matmuls so the PE has ~430ns of queued work before it needs the
    ps_s slot that exp() frees -- absorbing ACT-engine jitter.
  - all tensors use packed per-partition-contiguous DRAM layouts so each
    head is ONE fat input DMA per tensor and each (head, qblock) is ONE
    output DMA: few DMA instructions -> less Sync-engine serialization and
    a much shorter TileContext epilogue (which gates the measured
    exec_time via its trailing instruction-fetches).
  - a burst of warmup matmuls on a zeroed scratch tile keeps the PE
    continuously busy from engine-init until the first input tiles land,
    so the HAM clock gate (1.2 -> 2.4 GHz) releases before real work
    starts and is not reset by an idle gap.
"""

import sys

import numpy as np

for _p in ("/opt/trn_rl_repo",):
    if _p not in sys.path:
        sys.path.insert(0, _p)

B, H, S, D = 4, 8, 2048, 256
N_CORES = 8
HPC = (B * H) // N_CORES  # heads per core
SOFTMAX_SCALE = 1.0 / 16.0
N_WARMUP = 7

_compiled = {}


def _build(nh, s, d):
    import concourse.bacc as bacc
    import concourse.mybir as mybir
    import concourse.tile as tile

    f32 = mybir.dt.float32
    f16 = mybir.dt.float16

    KC = s // 128  # contraction (key) chunks
    QB = s // 512  # query blocks
    DC = d // 128  # head-dim chunks

    nc = bacc.Bacc("TRN2", debug=False, num_devices=N_CORES)
    # qT/kT are packed partition-major on the host:
    # qT[h, p, dc, col] = Q[h, col, dc*128 + p], so each partition's data is
    # one contiguous DC*s*2B run and a whole head loads as ONE DMA.
    qT = nc.dram_tensor("qT", [nh, 128, DC, s], f16, kind="ExternalInput")
    kT = nc.dram_tensor("kT", [nh, 128, DC, s], f16, kind="ExternalInput")
    vaw = d + 1  # ones col at d (softmax denominator rides along)
    # vA[h, p, kc, :] = V_aug[h, kc*128 + p, :] -- same packing trick.
    vA = nc.dram_tensor("vA", [nh, 128, KC, vaw], f16, kind="ExternalInput")
    # o[h, qb, p, qs*d + c] = out[h, qb*512 + qs*128 + p, c]: the four
    # 128-row lanes of a query block store side-by-side from one SBUF tile
    # in ONE DMA; the host permutes lanes back (cheap numpy reshape).
    o = nc.dram_tensor("o", [nh, QB, 128, 4 * d], f32, kind="ExternalOutput")

    with tile.TileContext(nc) as tc:
        with (
            tc.tile_pool(name="kt", bufs=2) as kt_pool,
            tc.tile_pool(name="qt", bufs=2) as qt_pool,
            tc.tile_pool(name="va", bufs=2) as va_pool,
            tc.tile_pool(name="exp", bufs=8) as exp_pool,
            tc.tile_pool(name="outp", bufs=2) as out_pool,
            tc.tile_pool(name="rec", bufs=4) as rec_pool,
            tc.tile_pool(name="warm", bufs=1) as warm_pool,
            tc.tile_pool(name="ps_s", bufs=2, space="PSUM") as ps_s_pool,
            tc.tile_pool(name="ps_o", bufs=6, space="PSUM") as ps_o_pool,
        ):
            # --- DMA emission (per head, first-use ordered) ---
            kts, qts, vas = [], [], []
            for h in range(nh):
                kt = kt_pool.tile([128, DC, s], f16, name=f"kt_{h}", tag="kt")
                qt = qt_pool.tile([128, DC, s], f16, name=f"qt_{h}", tag="qt")
                va = va_pool.tile([128, KC, vaw], f16, name=f"va_{h}", tag="va")
                kts.append(kt); qts.append(qt); vas.append(va)

            def emit_head_dma(h):
                kt, qt, va = kts[h], qts[h], vas[h]
                if h == 0:
                    # first-use ordered, finer-grained: the first score
                    # matmul needs only kt cols 0:128 and qt cols 0:512 --
                    # land those first so real matmuls can start ASAP, then
                    # feed kt chunks and va chunks in consumption order
                    # (attn@V lane 0 reads va[:, kc] two steps behind the
                    # kc-th score matmul).
                    # DMA completion waits are per-queue thresholds, so every
                    # DMA issued before the one a consumer needs delays that
                    # consumer. Different issuing engines have independent
                    # DGE queues/semaphores: split the critical first loads
                    # across Sync and Scalar (whose DGE is idle until the
                    # first exp at ~11us) so they transfer in parallel and
                    # nothing queues behind the big kt/va bulk.
                    nc.sync.dma_start(kt[:, :, 0:128], kT.ap()[h, :, :, 0:128])
                    nc.scalar.dma_start(qt[:, :, 0:512], qT.ap()[h, :, :, 0:512])
                    nc.scalar.dma_start(kt[:, :, 128:512], kT.ap()[h, :, :, 128:512])
                    nc.sync.dma_start(va[:, 0:2, :], vA.ap()[h, :, 0:2, :])
                    nc.sync.dma_start(va[:, 2:4, :], vA.ap()[h, :, 2:4, :])
                    nc.sync.dma_start(kt[:, :, 512:1024], kT.ap()[h, :, :, 512:1024])
                    nc.sync.dma_start(va[:, 4:8, :], vA.ap()[h, :, 4:8, :])
                    nc.sync.dma_start(kt[:, :, 1024:s], kT.ap()[h, :, :, 1024:s])
                    nc.sync.dma_start(va[:, 8:KC, :], vA.ap()[h, :, 8:KC, :])
                    nc.sync.dma_start(qt[:, :, 512:s], qT.ap()[h, :, :, 512:s])
                else:
                    nc.sync.dma_start(kt[:], kT.ap()[h])
                    nc.sync.dma_start(qt[:], qT.ap()[h])
                    nc.sync.dma_start(va[:], vA.ap()[h])

            # --- flat software pipeline over (head, qb, kc) ---
            # iteration t: scores(t) + exp(t); attn@V lane qs processes
            # iteration t-2-qs, so the four accumulator lanes finish (and
            # normalize + free their PSUM bank) one per iteration instead
            # of colliding at block boundaries.
            NIT = nh * QB * KC
            exps = [None] * NIT
            ps_os = {}
            osbs = {}

            def av_lane(t_av, qs):
                h, r = divmod(t_av, QB * KC)
                qb, kc = divmod(r, KC)
                po = ps_os[(h, qb)]
                nc.tensor.matmul(
                    po[qs][:],
                    exps[t_av][:, qs * 128:(qs + 1) * 128],
                    vas[h][:, kc, :],
                    start=(kc == 0),
                    stop=(kc == KC - 1),
                )
                if kc == KC - 1:
                    rec = rec_pool.tile([128, 1], f32, name=f"rec_{h}_{qb}_{qs}", tag="rec")
                    nc.vector.reciprocal(rec[:], po[qs][:, d:d + 1])
                    osb = osbs[(h, qb)]
                    last_block = h == nh - 1 and qb == QB - 1
                    if last_block:
                        # kernel tail: no exps remain, so spread the four
                        # normalize+store chains across three idle engines
                        # (Vector keeps only the reciprocals): lanes 0,2 run
                        # entirely on GpSimd, lanes 1,3 on Scalar, each
                        # storing its own 32KB sliver immediately -- the
                        # exit barrier behind the last store runs in the
                        # HAM-throttled window, so every ns earlier pays
                        # double.
                        # (GpSimd cannot read PSUM, so the muls stay on
                        # Vector/Scalar; lane 0-2 stores go via Sync, whose
                        # queue is empty during the drain -- a lane-1 store
                        # on Scalar would delay lane 3's mul behind it.)
                        eng = nc.scalar if qs == 3 else nc.sync
                        if qs % 2 == 1:
                            nc.scalar.mul(
                                osb[:, qs * d:(qs + 1) * d], po[qs][:, 0:d], rec[:]
                            )
                        else:
                            nc.vector.tensor_scalar_mul(
                                osb[:, qs * d:(qs + 1) * d], po[qs][:, 0:d], rec[:]
                            )
                        eng.dma_start(
                            o.ap()[h, qb, :, qs * d:(qs + 1) * d],
                            osb[:, qs * d:(qs + 1) * d],
                        )
                        if qs == 3:
                            ps_os.pop((h, qb))
                            osbs.pop((h, qb))
                    else:
                        nc.vector.tensor_scalar_mul(
                            osb[:, qs * d:(qs + 1) * d], po[qs][:, 0:d], rec[:]
                        )
                    if not last_block and qs == 3:
                        # mid-kernel: one fat store per block on the Sync
                        # queue (it has ~27us of slack).
                        nc.sync.dma_start(o.ap()[h, qb], osb[:])
                        ps_os.pop((h, qb))
                        osbs.pop((h, qb))

            # PE warmup: the HAM clock gate starts at 1.2 GHz and only
            # releases after a few us of SUSTAINED matmul activity (an idle
            # gap resets it). Burn the initial DMA wait on dummy matmuls
            # over a zeroed scratch tile, sized so the PE stays busy until
            # the first real input tiles have landed.
            wsrc = warm_pool.tile([128, 512], f16, name="wsrc")
            nc.vector.memset(wsrc[:], 0.0)
            for w in range(N_WARMUP):
                ps_w = ps_s_pool.tile([128, 512], f32, name=f"ps_w_{w}", tag="ps_s")
                nc.tensor.matmul(ps_w[:], wsrc[:, 0:128], wsrc[:], start=True, stop=True)

            emit_head_dma(0)
            for t in range(NIT):
                # scores+exp first: exp(t) starts as early as possible
                # within the step, maximizing slack for both the ps_s WAR
                # (scores(t+2)) and the attn@V weight load of step t+2.
                if True:
                    h, r = divmod(t, QB * KC)
                    qb, kc = divmod(r, KC)
                    if r == 0 and h + 1 < nh:
                        emit_head_dma(h + 1)  # prefetch next head
                    if kc == 0:
                        ps_os[(h, qb)] = [
                            ps_o_pool.tile([128, vaw], f32, name=f"ps_o_{h}_{qb}_{qs}", tag="ps_o")
                            for qs in range(4)
                        ]
                        osbs[(h, qb)] = out_pool.tile(
                            [128, 4 * d], f32, name=f"osb_{h}_{qb}", tag="outp"
                        )
                    ps_s = ps_s_pool.tile([128, 512], f32, name=f"ps_s_{h}_{qb}_{kc}", tag="ps_s")
                    for dc in range(DC):
                        nc.tensor.matmul(
                            ps_s[:],
                            kts[h][:, dc, kc * 128:(kc + 1) * 128],
                            qts[h][:, dc, qb * 512:(qb + 1) * 512],
                            start=(dc == 0),
                            stop=(dc == DC - 1),
                        )
                    expt = exp_pool.tile([128, 512], f16, name=f"expt_{h}_{qb}_{kc}", tag="exp")
                    nc.scalar.activation(
                        expt[:], ps_s[:], mybir.ActivationFunctionType.Exp,
                        scale=SOFTMAX_SCALE,
                    )
                    exps[t] = expt
                for qs in range(4):
                    t_av = t - 2 - qs
                    if 0 <= t_av < NIT:
                        av_lane(t_av, qs)
                if t >= 6 and t - 6 >= 0:
                    exps[t - 6] = None

            # drain: the remaining lane matmuls (lane qs is 2+qs iterations
            # behind) packed tightly instead of spread over 6 more skewed
            # steps -- all their exps exist, so the PE finishes ~1us sooner
            # and the final normalize/store (and the exit barrier behind it,
            # which runs in the HAM-throttled window) start earlier.
            for t_av in range(NIT - 5, NIT):
                for qs in range(4):
                    if t_av >= NIT - 2 - qs:
                        av_lane(t_av, qs)

    nc.compile()
    return nc


def _get_nc(nh, s, d):
    key = (nh, s, d)
    if key not in _compiled:
        _compiled[key] = _build(nh, s, d)
    return _compiled[key]


def _make_in_maps(queries, keys, values, n_cores):
    """queries/keys/values: [NHEADS_TOTAL, s, d] fp32 -> per-core input dicts."""
    nht, s, d = queries.shape
    nh = nht // n_cores
    dc = d // 128
    kc = s // 128
    pad = np.ones((nh, s, 1), dtype=np.float16)
    in_maps = []
    for c in range(n_cores):
        h0, h1 = c * nh, (c + 1) * nh
        # [nh, s, d] -> [nh, d, s] -> [nh, DC, 128, s] -> [nh, 128, DC, s]
        qp = (queries[h0:h1].transpose(0, 2, 1)
              .reshape(nh, dc, 128, s).transpose(0, 2, 1, 3))
        kp = (keys[h0:h1].transpose(0, 2, 1)
              .reshape(nh, dc, 128, s).transpose(0, 2, 1, 3))
        in_maps.append({
            "qT": np.ascontiguousarray(qp).astype(np.float16),
            "kT": np.ascontiguousarray(kp).astype(np.float16),
            "vA": np.ascontiguousarray(
                np.concatenate([values[h0:h1].astype(np.float16), pad], axis=2)
                .reshape(nh, kc, 128, -1).transpose(0, 2, 1, 3)),
        })
    return in_maps


def _run(queries, keys, values, n_cores):
    """queries/keys/values: [NHEADS_TOTAL, s, d] fp32. Returns [NHEADS_TOTAL, s, d]."""
    from concourse import bass_utils

    nht, s, d = queries.shape
    nh = nht // n_cores
    qb = s // 512
    nc = _get_nc(nh, s, d)

    in_maps = _make_in_maps(queries, keys, values, n_cores)
    res = bass_utils.run_bass_kernel_spmd(nc, in_maps, core_ids=list(range(n_cores)))
    out = np.empty((nht, s, d), dtype=np.float32)
    for c in range(n_cores):
        # o[h, qb, p, qs*d+c] -> rows qb*512 + qs*128 + p
        r = res.results[c]["o"].reshape(nh, qb, 128, 4, d)
        out[c * nh:(c + 1) * nh] = (
            r.transpose(0, 1, 3, 2, 4).reshape(nh, s, d)
        )
    return out


def kernel(queries, keys, values, adj=None):
    queries = np.asarray(queries, dtype=np.float32)
    keys = np.asarray(keys, dtype=np.float32)
    values = np.asarray(values, dtype=np.float32)
    b, h, s, d = queries.shape
    out = _run(
        queries.reshape(b * h, s, d),
        keys.reshape(b * h, s, d),
        values.reshape(b * h, s, d),
        N_CORES,
    )
    # reference returns a raw reshape of the contiguous [B,H,S,D] result
    return out.reshape(s, b, h, d)


# revision 25
# speedup vs baseline: 1.0065x; 1.0041x over previous
"""Trainium2 Bass kernel: batched multi-head dot-product attention.

Full-size problem: queries/keys/values [B=4, H=8, S=2048, D=256] fp32,
out = softmax(Q K^T / 16) V, returned reshaped to (S, B, H, D).

Sharding: the 32 (B*H) heads are split across 8 NeuronCores, 4 heads per
core; each core computes full attention for its heads (no cross-core
communication).

Per-head algorithm (per 512-query block):
  - scores are computed TRANSPOSED (keys on the partition dim, queries on
    the free dim): psum_sT[k, q] = sum_d KT[d, k] * QT[d, q], so that after
    exp() the attention weights are already laid out as the stationary
    (lhsT) operand of the attn @ V matmul -- no on-chip transposes needed.
  - softmax skips the max subtraction: scores/16 are ~N(0,1), exp cannot
    overflow fp32, and jax.nn.softmax's max shift is mathematically a
    no-op. The 1/16 scale is folded into the Exp activation.
  - the softmax denominator falls out of the attn @ V matmul for free: V
    is augmented host-side with a ones column, so column D of the output
    accumulator is sum_k exp(score) per query. A reciprocal + per-partition
    scaled multiply on the Vector engine normalizes while evacuating PSUM.
  - matmuls run in fp16 (inputs converted host-side; exp output written as
    fp16 by the Scalar engine; accumulation stays fp32 in PSUM) for
    full-rate streaming and fast weight loads; measured output rel err vs
    the fp32 reference is ~6e-4.
  - one flat software pipeline over (head, qblock, kchunk): scores + exp
    run 2+ iterations ahead of the attn@V matmuls, and the four PSUM
    accumulator lanes are skewed one iteration apart so their
    normalize+store chains stagger instead of colliding at block
    boundaries. Within each step the attn@V lanes are emitted BEFORE the
    scores matmuls so the PE has ~430ns of queued work before it needs the
    ps_s slot that exp() frees -- absorbing ACT-engine jitter.
  - all tensors use packed per-partition-contiguous DRAM layouts so each
    head is ONE fat input DMA per tensor and each (head, qblock) is ONE
    output DMA: few DMA instructions -> less Sync-engine serialization and
    a much shorter TileContext epilogue (which gates the measured
    exec_time via its trailing instruction-fetches).
  - a burst of warmup matmuls on a zeroed scratch tile keeps the PE
    continuously busy from engine-init until the first input tiles land,
    so the HAM clock gate (1.2 -> 2.4 GHz) releases before real work
    starts and is not reset by an idle gap.
"""

import sys

import numpy as np

for _p in ("/opt/trn_rl_repo",):
    if _p not in sys.path:
        sys.path.insert(0, _p)

B, H, S, D = 4, 8, 2048, 256
N_CORES = 8
HPC = (B * H) // N_CORES  # heads per core
SOFTMAX_SCALE = 1.0 / 16.0
N_WARMUP = 7

_compiled = {}


def _build(nh, s, d):
    import concourse.bacc as bacc
    import concourse.mybir as mybir
    import concourse.tile as tile

    f32 = mybir.dt.float32
    f16 = mybir.dt.float16

    KC = s // 128  # contraction (key) chunks
    QB = s // 512  # query blocks
    DC = d // 128  # head-dim chunks

    nc = bacc.Bacc("TRN2", debug=False, num_devices=N_CORES)
    # qT/kT are packed partition-major on the host:
    # qT[h, p, dc, col] = Q[h, col, dc*128 + p], so each partition's data is
    # one contiguous DC*s*2B run and a whole head loads as ONE DMA.
    qT = nc.dram_tensor("qT", [nh, 128, DC, s], f16, kind="ExternalInput")
    kT = nc.dram_tensor("kT", [nh, 128, DC, s], f16, kind="ExternalInput")
    vaw = d + 1  # ones col at d (softmax denominator rides along)
    # vA[h, p, kc, :] = V_aug[h, kc*128 + p, :] -- same packing trick.
    vA = nc.dram_tensor("vA", [nh, 128, KC, vaw], f16, kind="ExternalInput")
    # o[h, qb, p, qs*d + c] = out[h, qb*512 + qs*128 + p, c]: the four
    # 128-row lanes of a query block store side-by-side from one SBUF tile
    # in ONE DMA; the host permutes lanes back (cheap numpy reshape).
    o = nc.dram_tensor("o", [nh, QB, 128, 4 * d], f32, kind="ExternalOutput")

    with tile.TileContext(nc) as tc:
        with (
            tc.tile_pool(name="kt", bufs=2) as kt_pool,
            tc.tile_pool(name="qt", bufs=2) as qt_pool,
            tc.tile_pool(name="va", bufs=2) as va_pool,
            tc.tile_pool(name="exp", bufs=8) as exp_pool,
            tc.tile_pool(name="outp", bufs=2) as out_pool,
            tc.tile_pool(name="rec", bufs=4) as rec_pool,
            tc.tile_pool(name="warm", bufs=1) as warm_pool,
            tc.tile_pool(name="ps_s", bufs=2, space="PSUM") as ps_s_pool,
            tc.tile_pool(name="ps_o", bufs=6, space="PSUM") as ps_o_pool,
        ):
            # --- DMA emission (per head, first-use ordered) ---
            kts, qts, vas = [], [], []
            for h in range(nh):
                kt = kt_pool.tile([128, DC, s], f16, name=f"kt_{h}", tag="kt")
                qt = qt_pool.tile([128, DC, s], f16, name=f"qt_{h}", tag="qt")
                va = va_pool.tile([128, KC, vaw], f16, name=f"va_{h}", tag="va")
                kts.append(kt); qts.append(qt); vas.append(va)

            def emit_head_dma(h):
                kt, qt, va = kts[h], qts[h], vas[h]
                if h == 0:
                    # first-use ordered, finer-grained: the first score
                    # matmul needs only kt cols 0:128 and qt cols 0:512 --
                    # land those first so real matmuls can start ASAP, then
                    # feed kt chunks and va chunks in consumption order
                    # (attn@V lane 0 reads va[:, kc] two steps behind the
                    # kc-th score matmul).
                    # DMA completion waits are per-queue thresholds, so every
                    # DMA issued before the one a consumer needs delays that
                    # consumer. Different issuing engines have independent
                    # DGE queues/semaphores: split the critical first loads
                    # across Sync and Scalar (whose DGE is idle until the
                    # first exp at ~11us) so they transfer in parallel and
                    # nothing queues behind the big kt/va bulk.
                    nc.sync.dma_start(kt[:, :, 0:128], kT.ap()[h, :, :, 0:128])
                    nc.scalar.dma_start(qt[:, :, 0:512], qT.ap()[h, :, :, 0:512])
                    # iteration 1 needs only cols 128:256 -- give it its own
                    # small transfer so it isn't gated by the 256:512 bulk
                    nc.scalar.dma_start(kt[:, :, 128:256], kT.ap()[h, :, :, 128:256])
                    nc.scalar.dma_start(kt[:, :, 256:512], kT.ap()[h, :, :, 256:512])
                    nc.sync.dma_start(va[:, 0:2, :], vA.ap()[h, :, 0:2, :])
                    nc.sync.dma_start(va[:, 2:4, :], vA.ap()[h, :, 2:4, :])
                    nc.sync.dma_start(kt[:, :, 512:1024], kT.ap()[h, :, :, 512:1024])
                    nc.sync.dma_start(va[:, 4:8, :], vA.ap()[h, :, 4:8, :])
                    nc.sync.dma_start(kt[:, :, 1024:s], kT.ap()[h, :, :, 1024:s])
                    nc.sync.dma_start(va[:, 8:KC, :], vA.ap()[h, :, 8:KC, :])
                    nc.sync.dma_start(qt[:, :, 512:s], qT.ap()[h, :, :, 512:s])
                else:
                    nc.sync.dma_start(kt[:], kT.ap()[h])
                    nc.sync.dma_start(qt[:], qT.ap()[h])
                    nc.sync.dma_start(va[:], vA.ap()[h])

            # --- flat software pipeline over (head, qb, kc) ---
            # iteration t: scores(t) + exp(t); attn@V lane qs processes
            # iteration t-2-qs, so the four accumulator lanes finish (and
            # normalize + free their PSUM bank) one per iteration instead
            # of colliding at block boundaries.
            NIT = nh * QB * KC
            exps = [None] * NIT
            ps_os = {}
            osbs = {}

            def av_lane(t_av, qs):
                h, r = divmod(t_av, QB * KC)
                qb, kc = divmod(r, KC)
                po = ps_os[(h, qb)]
                nc.tensor.matmul(
                    po[qs][:],
                    exps[t_av][:, qs * 128:(qs + 1) * 128],
                    vas[h][:, kc, :],
                    start=(kc == 0),
                    stop=(kc == KC - 1),
                )
                if kc == KC - 1:
                    rec = rec_pool.tile([128, 1], f32, name=f"rec_{h}_{qb}_{qs}", tag="rec")
                    nc.vector.reciprocal(rec[:], po[qs][:, d:d + 1])
                    osb = osbs[(h, qb)]
                    last_block = h == nh - 1 and qb == QB - 1
                    if last_block:
                        # kernel tail: no exps remain, so spread the four
                        # normalize+store chains across three idle engines
                        # (Vector keeps only the reciprocals): lanes 0,2 run
                        # entirely on GpSimd, lanes 1,3 on Scalar, each
                        # storing its own 32KB sliver immediately -- the
                        # exit barrier behind the last store runs in the
                        # HAM-throttled window, so every ns earlier pays
                        # double.
                        # (GpSimd cannot read PSUM, so the muls stay on
                        # Vector/Scalar; lane 0-2 stores go via Sync, whose
                        # queue is empty during the drain -- a lane-1 store
                        # on Scalar would delay lane 3's mul behind it.)
                        eng = nc.scalar if qs == 3 else nc.sync
                        if qs % 2 == 1:
                            nc.scalar.mul(
                                osb[:, qs * d:(qs + 1) * d], po[qs][:, 0:d], rec[:]
                            )
                        else:
                            nc.vector.tensor_scalar_mul(
                                osb[:, qs * d:(qs + 1) * d], po[qs][:, 0:d], rec[:]
                            )
                        eng.dma_start(
                            o.ap()[h, qb, :, qs * d:(qs + 1) * d],
                            osb[:, qs * d:(qs + 1) * d],
                        )
                        if qs == 3:
                            ps_os.pop((h, qb))
                            osbs.pop((h, qb))
                    else:
                        nc.vector.tensor_scalar_mul(
                            osb[:, qs * d:(qs + 1) * d], po[qs][:, 0:d], rec[:]
                        )
                    if not last_block and qs == 3:
                        # mid-kernel: one fat store per block on the Sync
                        # queue (it has ~27us of slack).
                        nc.sync.dma_start(o.ap()[h, qb], osb[:])
                        ps_os.pop((h, qb))
                        osbs.pop((h, qb))

            # PE warmup: the HAM clock gate starts at 1.2 GHz and only
            # releases after a few us of SUSTAINED matmul activity (an idle
            # gap resets it). Burn the initial DMA wait on dummy matmuls
            # over a zeroed scratch tile, sized so the PE stays busy until
            # the first real input tiles have landed.
            wsrc = warm_pool.tile([128, 512], f16, name="wsrc")
            nc.vector.memset(wsrc[:], 0.0)
            for w in range(N_WARMUP):
                ps_w = ps_s_pool.tile([128, 512], f32, name=f"ps_w_{w}", tag="ps_s")
                nc.tensor.matmul(ps_w[:], wsrc[:, 0:128], wsrc[:], start=True, stop=True)

            emit_head_dma(0)
            for t in range(NIT):
                # scores+exp first: exp(t) starts as early as possible
                # within the step, maximizing slack for both the ps_s WAR
                # (scores(t+2)) and the attn@V weight load of step t+2.
                if True:
                    h, r = divmod(t, QB * KC)
                    qb, kc = divmod(r, KC)
                    if r == 0 and h + 1 < nh:
                        emit_head_dma(h + 1)  # prefetch next head
                    if kc == 0:
                        ps_os[(h, qb)] = [
                            ps_o_pool.tile([128, vaw], f32, name=f"ps_o_{h}_{qb}_{qs}", tag="ps_o")
                            for qs in range(4)
                        ]
                        osbs[(h, qb)] = out_pool.tile(
                            [128, 4 * d], f32, name=f"osb_{h}_{qb}", tag="outp"
                        )
                    ps_s = ps_s_pool.tile([128, 512], f32, name=f"ps_s_{h}_{qb}_{kc}", tag="ps_s")
                    for dc in range(DC):
                        nc.tensor.matmul(
                            ps_s[:],
                            kts[h][:, dc, kc * 128:(kc + 1) * 128],
                            qts[h][:, dc, qb * 512:(qb + 1) * 512],
                            start=(dc == 0),
                            stop=(dc == DC - 1),
                        )
                    expt = exp_pool.tile([128, 512], f16, name=f"expt_{h}_{qb}_{kc}", tag="exp")
                    nc.scalar.activation(
                        expt[:], ps_s[:], mybir.ActivationFunctionType.Exp,
                        scale=SOFTMAX_SCALE,
                    )
                    exps[t] = expt
                for qs in range(4):
                    t_av = t - 2 - qs
                    if 0 <= t_av < NIT:
                        av_lane(t_av, qs)
                if t >= 6 and t - 6 >= 0:
                    exps[t - 6] = None

            # drain: the remaining lane matmuls (lane qs is 2+qs iterations
            # behind) packed tightly instead of spread over 6 more skewed
            # steps -- all their exps exist, so the PE finishes ~1us sooner
            # and the final normalize/store (and the exit barrier behind it,
            # which runs in the HAM-throttled window) start earlier.
            for t_av in range(NIT - 5, NIT):
                for qs in range(4):
                    if t_av >= NIT - 2 - qs:
                        av_lane(t_av, qs)

    nc.compile()
    return nc


def _get_nc(nh, s, d):
    key = (nh, s, d)
    if key not in _compiled:
        _compiled[key] = _build(nh, s, d)
    return _compiled[key]


def _make_in_maps(queries, keys, values, n_cores):
    """queries/keys/values: [NHEADS_TOTAL, s, d] fp32 -> per-core input dicts."""
    nht, s, d = queries.shape
    nh = nht // n_cores
    dc = d // 128
    kc = s // 128
    pad = np.ones((nh, s, 1), dtype=np.float16)
    in_maps = []
    for c in range(n_cores):
        h0, h1 = c * nh, (c + 1) * nh
        # [nh, s, d] -> [nh, d, s] -> [nh, DC, 128, s] -> [nh, 128, DC, s]
        qp = (queries[h0:h1].transpose(0, 2, 1)
              .reshape(nh, dc, 128, s).transpose(0, 2, 1, 3))
        kp = (keys[h0:h1].transpose(0, 2, 1)
              .reshape(nh, dc, 128, s).transpose(0, 2, 1, 3))
        in_maps.append({
            "qT": np.ascontiguousarray(qp).astype(np.float16),
            "kT": np.ascontiguousarray(kp).astype(np.float16),
            "vA": np.ascontiguousarray(
                np.concatenate([values[h0:h1].astype(np.float16), pad], axis=2)
                .reshape(nh, kc, 128, -1).transpose(0, 2, 1, 3)),
        })
    return in_maps


def _run(queries, keys, values, n_cores):
    """queries/keys/values: [NHEADS_TOTAL, s, d] fp32. Returns [NHEADS_TOTAL, s, d]."""
    from concourse import bass_utils

    nht, s, d = queries.shape
    nh = nht // n_cores
    qb = s // 512
    nc = _get_nc(nh, s, d)

    in_maps = _make_in_maps(queries, keys, values, n_cores)
    res = bass_utils.run_bass_kernel_spmd(nc, in_maps, core_ids=list(range(n_cores)))
    out = np.empty((nht, s, d), dtype=np.float32)
    for c in range(n_cores):
        # o[h, qb, p, qs*d+c] -> rows qb*512 + qs*128 + p
        r = res.results[c]["o"].reshape(nh, qb, 128, 4, d)
        out[c * nh:(c + 1) * nh] = (
            r.transpose(0, 1, 3, 2, 4).reshape(nh, s, d)
        )
    return out


def kernel(queries, keys, values, adj=None):
    queries = np.asarray(queries, dtype=np.float32)
    keys = np.asarray(keys, dtype=np.float32)
    values = np.asarray(values, dtype=np.float32)
    b, h, s, d = queries.shape
    out = _run(
        queries.reshape(b * h, s, d),
        keys.reshape(b * h, s, d),
        values.reshape(b * h, s, d),
        N_CORES,
    )
    # reference returns a raw reshape of the contiguous [B,H,S,D] result
    return out.reshape(s, b, h, d)
